# revision 1
# baseline (speedup 1.0000x reference)
"""Distributed kNN retrieval (MemoryBank) on 8 Trainium2 NeuronCores.

Problem: B=2048 queries, N=100000 keys, D=256; cosine top-5, softmax(T=0.1)
weights, weighted sum of the normalized key vectors, distances = 1 - top sim.

Strategy (data-parallel over memory rows, per the standard distributed kNN
recipe):
  * Host: l2-normalize queries and keys, shard keys into 8x12500 rows,
    transpose each shard to [D, Nshard] (bf16) and pad to 12544 columns.
  * Device (SPMD, identical program on 8 cores): sim = qnT.T @ memT via
    bf16 TensorE matmuls (PSUM fp32), then a 16-wide windowed max over the
    key axis on VectorE -> wmax [2048, 784] per core.  Windowed maxes are
    a provably sufficient statistic: any key in the global top-5 must lie
    in one of the top-(5+1) windows globally (the +1 covers the excluded
    key), so the top-8 windows always contain the exact answer.
  * Host: global top-8 windows per query across the 8*784 window maxes,
    gather those <=128 candidate keys, rescore them exactly in fp32/fp64,
    drop the excluded id, take top-5, softmax, weighted sum, normalize.

bf16 on the similarity path only affects *candidate window selection*
(rounding errors average out over D=256: sim error ~1e-4, while window-max
gaps at the selection margin are ~1e-2); final scores come from the exact
host rescore, so outputs match the fp32 reference to ~1e-7 rel.
"""
import numpy as np
import ml_dtypes

# ---------------------------------------------------------------------------
# Compat shims for the toolchain in this container.
# ---------------------------------------------------------------------------
import bass_rust
import concourse.mybir as mybir
import concourse.bass as bass
import concourse.tile as tile
from concourse.vector_clock import ScopedClock
from concourse.bass_utils import run_bass_kernel_spmd
from contextlib import ExitStack

_MAX_WAITS = 1


def _patched_drain_and_barrier(self, tick_clock, wait_clock):
    # walrus here rejects TPB_CTRL instructions carrying >1 sem wait; absorb
    # the tile tail-drain waits into SP nops (split one wait per nop below).
    nc = self.nc
    nop_inst = nc.sync.nop(nofuse=True)
    wait_clock.add_sem_waits(nop_inst.ins, ScopedClock({None: tick_clock.global_clock}))
    nc.sync.drain()
    nc.all_engine_barrier()
    assert self.sems is not None
    popped = nc._tile_sem_poison_stack.pop()
    assert popped is self._sem_poison
    nc.clear_and_free_semaphores(list(self.sems.allocated().values()))
    nc.all_engine_barrier()


tile.TileContext._drain_and_barrier = _patched_drain_and_barrier


def _split_waits(nc, max_waits=_MAX_WAITS):
    """No instruction may carry more than max_waits sem waits: hoist the
    excess onto same-engine NoOps inserted just before it."""
    n_split = 0
    for name, bbw in nc.bb_map.items():
        bb = bbw.bb
        out = []
        for inst in bb.instructions:
            si = inst.sync_info
            if si is not None and len(si.on_wait) > max_waits:
                waits = list(si.on_wait)
                updates = list(si.on_update)
                extra, keep = waits[:-max_waits], waits[-max_waits:]
                for w in extra:
                    nop = mybir.InstNoOp(name=f"{inst.name}-wsplit{n_split}",
                                         ins=[], outs=[])
                    nop.engine = inst.engine
                    nop.sync_info = bass_rust.SyncInfo(on_wait=[w], on_update=[])
                    out.append(nop)
                    n_split += 1
                inst.sync_info = bass_rust.SyncInfo(on_wait=keep, on_update=updates)
            out.append(inst)
        bb.instructions = out
    return n_split


# ---------------------------------------------------------------------------
# Problem constants (hardcoded per the harness contract).
# ---------------------------------------------------------------------------
B, D, N = 2048, 256, 100000
NCORES = 8
NSHARD = N // NCORES            # 12500 keys per core
NPAD = 12544                    # 784 windows * 16
WIN = 16
QT = B // 128                   # 16 query tiles of 128
NWOUT = NPAD // WIN             # 784 window maxes per query per core
TOP_K = 5
TEMPERATURE = 0.1
EPS = 1e-12
TOPW = 8                        # windows kept per query at the host merge

F32 = mybir.dt.float32
BF16 = mybir.dt.bfloat16

REG_ORDER = [6, 0, 1, 2, 3, 4, 5]          # small region first: early start
REG_LEN = {r: (2048 if r < 6 else 256) for r in range(7)}


def _build_nc():
    nc = bass.Bass("TRN2", target_bir_lowering=False, debug=False,
                   num_devices=NCORES)
    qnT = nc.dram_tensor("qnT", [2, 128, B], BF16, kind="ExternalInput").ap()
    memT = nc.dram_tensor("memT", [2, 128, NPAD], BF16, kind="ExternalInput").ap()
    o_wmax = nc.dram_tensor("wmax", [B, NWOUT], F32, kind="ExternalOutput").ap()

    with tile.TileContext(nc) as tc:
        with ExitStack() as ctx:
            p_mem = ctx.enter_context(tc.tile_pool(name="mem", bufs=1))
            p_qt = ctx.enter_context(tc.tile_pool(name="qt", bufs=1))
            p_wm = ctx.enter_context(tc.tile_pool(name="wm", bufs=2))
            p_psum = ctx.enter_context(tc.tile_pool(name="psum", bufs=4,
                                                    space="PSUM"))

            # per-(kc, region) memory tiles; DMAs split into 512-col pieces
            # and issued in PE processing order so the first matmul starts
            # as soon as the first small region lands.
            mem = {}
            for r in REG_ORDER:
                for kc in (0, 1):
                    mem[(kc, r)] = p_mem.tile([128, REG_LEN[r]], BF16,
                                              tag=f"mem{kc}_{r}",
                                              name=f"mem{kc}_{r}")
            qt = {kc: p_qt.tile([128, B], BF16, tag=f"qt{kc}", name=f"qt{kc}")
                  for kc in (0, 1)}
            qt0 = {kc: p_qt.tile([128, 128], BF16, tag=f"qt0_{kc}",
                                 name=f"qt0_{kc}") for kc in (0, 1)}
            for kc in (0, 1):
                nc.sync.dma_start(qt0[kc][:], qnT[kc, :, 0:128])
            for r in REG_ORDER:
                off = r * 2048
                for kc in (0, 1):
                    for j in range(REG_LEN[r] // 512 or 1):
                        w0 = j * 512
                        w1 = min(REG_LEN[r], w0 + 512)
                        nc.sync.dma_start(mem[(kc, r)][:, w0:w1],
                                          memT[kc, :, off + w0:off + w1])
            for kc in (0, 1):
                for j in range(4):
                    nc.sync.dma_start(qt[kc][:, j * 512:(j + 1) * 512],
                                      qnT[kc, :, j * 512:(j + 1) * 512])

            for t in range(QT):
                wm = p_wm.tile([128, NWOUT], F32, tag="wm")
                # 1024-wide PSUM chunks (2 banks, 4 bufs) keep TensorE ahead
                # of the VectorE windowed reduce.
                for r in REG_ORDER:
                    rl = REG_LEN[r]
                    for h in range(max(1, rl // 1024)):
                        cl = min(1024, rl)
                        ps = p_psum.tile([128, cl], F32, tag="ps")
                        for kc in (0, 1):
                            lhsT = (qt0[kc][:] if t == 0 else
                                    qt[kc][:, t * 128:(t + 1) * 128])
                            for s in range((cl + 511) // 512):
                                slen = min(512, cl - s * 512)
                                o0 = h * 1024 + s * 512
                                nc.tensor.matmul(
                                    ps[:, s * 512:s * 512 + slen],
                                    lhsT,
                                    mem[(kc, r)][:, o0:o0 + slen],
                                    start=(kc == 0), stop=(kc == 1))
                        w0 = r * 128 + h * 64
                        nc.vector.tensor_reduce(
                            wm[:, w0: w0 + cl // WIN],
                            ps[:].rearrange("p (w i) -> p w i", i=WIN),
                            axis=mybir.AxisListType.X, op=mybir.AluOpType.max)
                nc.sync.dma_start(o_wmax[t * 128:(t + 1) * 128, :], wm[:])
    _split_waits(nc)
    return nc


_NC_CACHE = None


def _get_nc():
    global _NC_CACHE
    if _NC_CACHE is None:
        _NC_CACHE = _build_nc()
    return _NC_CACHE


def kernel(query, keys, index_map, exclude_indices):
    query = np.asarray(query, np.float32)
    keys = np.asarray(keys, np.float32)
    index_map = np.asarray(index_map).astype(np.int64)
    excl = np.asarray(exclude_indices).astype(np.int64)

    # ---- host prep: normalize, shard, transpose, cast ----
    qn = query / np.maximum(np.sqrt((query * query).sum(1, keepdims=True)), EPS)
    kn = keys / np.maximum(np.sqrt((keys * keys).sum(1, keepdims=True)), EPS)
    qnT_bf = np.ascontiguousarray(qn.T).reshape(2, 128, B).astype(ml_dtypes.bfloat16)
    in_maps = []
    for c in range(NCORES):
        sht = np.ascontiguousarray(kn[c * NSHARD:(c + 1) * NSHARD].T)
        mt = np.zeros((2, 128, NPAD), ml_dtypes.bfloat16)
        mt[0, :, :NSHARD] = sht[:128].astype(ml_dtypes.bfloat16)
        mt[1, :, :NSHARD] = sht[128:].astype(ml_dtypes.bfloat16)
        in_maps.append({"qnT": qnT_bf, "memT": mt})

    # ---- device: per-core window maxes ----
    nc = _get_nc()
    res = run_bass_kernel_spmd(nc, in_maps, core_ids=list(range(NCORES)))
    results = res.results

    # ---- host merge: top-8 windows globally, exact rescore, finalize ----
    wmax = np.concatenate([r["wmax"] for r in results], axis=1)   # [B, 8*784]
    part = np.argpartition(-wmax, TOPW - 1, axis=1)[:, :TOPW]     # [B, TOPW]
    c_id, w_id = part // NWOUT, part % NWOUT
    lcols = (w_id[:, :, None] * WIN + np.arange(WIN)[None, None, :])  # [B,TOPW,16]
    valid = lcols < NSHARD
    cand = (c_id[:, :, None] * NSHARD + np.minimum(lcols, NSHARD - 1)).reshape(B, -1)
    valid = valid.reshape(B, -1)
    valid &= index_map[cand] != excl[:, None]

    cv = kn[cand]                                                 # [B, 128, 256]
    s = np.einsum('bd,bkd->bk', qn, cv, optimize=True)            # exact fp32
    s = np.where(valid, s.astype(np.float64), -np.inf)
    order = np.argsort(-s, axis=1, kind="stable")[:, :TOP_K]
    top_s = np.take_along_axis(s, order, axis=1)                  # [B, 5] desc
    top_i = np.take_along_axis(cand, order, axis=1)
    z = top_s / TEMPERATURE
    z = z - z.max(1, keepdims=True)
    e = np.exp(z)
    w = e / e.sum(1, keepdims=True)
    tv = kn[top_i].astype(np.float64)                             # [B, 5, 256]
    ret = (w[:, :, None] * tv).sum(1)
    ret = ret / np.maximum(np.sqrt((ret * ret).sum(1, keepdims=True)), EPS)
    dist = 1.0 - top_s[:, 0]
    return (ret.astype(np.float32), dist.astype(np.float32),
            w.astype(np.float32))
